# revision 64
# baseline (speedup 1.0000x reference)
"""Distributed causal multi-head attention for Trainium2 (8 NeuronCores).

Problem: B=8, S=1024, D=768, H=12, DH=64 causal MHA (dense_transformer).
Sharding: pure data parallel — batch element b runs on core b; weights are
replicated. No collectives.

Per-core kernel (bf16 TensorE compute, f32 PSUM accumulation):
  1. Startup is HBM-stream-bound, so only the tensors that gate compute are
     loaded up front, in arrival order: x rows, then W_Q/W_K rows 0-2, all
     W_V rows, W_Q/W_K rows 3-5 — all f32 via HWDGE on one queue, transposed
     on the PE (streamed row-group by row-group, interleaved with the first
     QKT chains so head-pair work starts as each row lands).  W_O loads
     mid-run (12 contiguous per-head [D,DH] chunks, last in the sync queue)
     and is PE-transposed as pair-1..3 fillers.  HAM clock warm-up: an
     ident-chain + short sprinkle matmuls bridge the DMA-gated stretches
     (PE-transpose-mode ops don't count as PE-busy for the clock gate).
  2. QKV projections on TensorE -> QT/KT [n,s] (transposed) and V [s,n],
     PSUM evictions alternating between VectorE and ScalarE, transpose
     batches alternating between two PSUM pools (ring depth 4) so
     evictions never pace the PE.
  3. Scores computed transposed per head: SC[q,p] = sum_h K[q,h] Q[p,h],
     two heads in alternating 64-row groups (concurrent PE sub-arrays),
     skipping causally-dead 128-col blocks; exp(SC/8) on ScalarE covers
     only causally-live columns (one merged 2-head exp for j>=4).  Masked
     column prefixes of the persistent double-buffered E tiles are zeroed
     once at startup; the diagonal 128x128 block is masked by multiplying
     with an upper-triangular tile on DVE (NEVER gpsimd: mixing op families
     there forces ~7us Q7 LIBRARY_RELOAD stalls).
  4. z^T = V^T E accumulated on TensorE with a ones-column per head riding
     the same matmul to produce softmax denominators; normalize with
     copy->reciprocal_approx_fast (PSUM-direct custom-DVE reads are
     broken) + gpsimd partition_broadcast (source must sit on partition 0)
     + VectorE multiply.
  5. Output projection from zT/woT tiles: rows 0-1 run as PE filler inside
     head-pair 5 (they only need ZT[:, 0:512]); pair 5's c1 is split into
     two 256-col z chains so rows 2-7 start as their ZT columns finish;
     tail evictions go to the then-idle ScalarE and one full-row DMA per
     128-row block minimizes the final sync-queue drain.
"""
import numpy as np

import concourse.bacc as bacc
import concourse.mybir as mybir
import concourse.tile as tile
from concourse.masks import make_identity, make_upper_triangular
from concourse.bass_utils import run_bass_kernel_spmd

f32 = mybir.dt.float32
bf16 = mybir.dt.bfloat16

B = 8
S, D, H, DH = 1024, 768, 12, 64
NT = 6    # n 128-tiles (head pairs)
MT = 6    # m 128-tiles
ST = 8    # s 128-tiles
PC = 2    # p chunks of 512
SCALE = 0.125  # 1/sqrt(DH)
W65 = DH + 1   # per-head V columns incl the ones column

N_CORES = 8


def build(n_cores: int = N_CORES, debug: bool = False):
    nc = bacc.Bacc("TRN2", target_bir_lowering=False, debug=False, num_devices=n_cores,
                   num_swdge_queues=4)

    x = nc.dram_tensor("x", [S, D], f32, kind="ExternalInput")
    W_Q = nc.dram_tensor("W_Q", [H, DH, D], f32, kind="ExternalInput")
    W_K = nc.dram_tensor("W_K", [H, DH, D], f32, kind="ExternalInput")
    W_V = nc.dram_tensor("W_V", [H, DH, D], f32, kind="ExternalInput")
    W_O = nc.dram_tensor("W_O", [H, D, DH], f32, kind="ExternalInput")
    out = nc.dram_tensor("out", [S, D], f32, kind="ExternalOutput")

    warmout = nc.dram_tensor("warmout", [1, 512], f32)

    dbg = {}
    if debug:
        for t in range(NT):
            dbg[f"dZT{t}"] = nc.dram_tensor(f"dZT{t}", [128, S], f32, kind="ExternalOutput")
            dbg[f"dQT{t}"] = nc.dram_tensor(f"dQT{t}", [128, S], f32, kind="ExternalOutput")
            dbg[f"dKT{t}"] = nc.dram_tensor(f"dKT{t}", [128, S], f32, kind="ExternalOutput")
            dbg[f"dwoT{t}"] = nc.dram_tensor(f"dwoT{t}", [128, D], f32, kind="ExternalOutput")
        for j in range(ST):
            dbg[f"dV{j}"] = nc.dram_tensor(f"dV{j}", [128, H * W65], f32, kind="ExternalOutput")
            dbg[f"dE{j}"] = nc.dram_tensor(f"dE{j}", [128, 2048 if j <= 3 else 1024], bf16,
                                           kind="ExternalOutput")

    with tile.TileContext(nc) as tc:
        from contextlib import ExitStack
        with ExitStack() as ctx:
            persist = ctx.enter_context(tc.tile_pool(name="persist", bufs=1))
            xstage = ctx.enter_context(tc.tile_pool(name="xstage", bufs=8))
            outsb_pool = ctx.enter_context(tc.tile_pool(name="outsb", bufs=2))
            small = ctx.enter_context(tc.tile_pool(name="small", bufs=2))
            ps_mm = ctx.enter_context(tc.tile_pool(name="ps_mm", bufs=2, space="PSUM"))
            ps_sc = ctx.enter_context(tc.tile_pool(name="ps_sc", bufs=2, space="PSUM"))
            ps_zt = ctx.enter_context(tc.tile_pool(name="ps_zt", bufs=2, space="PSUM"))

            # ---- init: DVE handles memsets (keeps the gpsimd queue nearly
            # empty — only ident/tri build + the z broadcasts live there) ----
            warm_src = persist.tile([128, 512], bf16, tag="warm", name="warm")
            nc.vector.memset(warm_src[:], 1.0)
            ident = persist.tile([128, 128], f32, tag="ident", name="ident")
            make_identity(nc, ident[:])
            tri2 = persist.tile([128, 256], bf16, tag="tri", name="tri")
            for half in range(2):
                make_upper_triangular(nc, tri2[:, half * 128:(half + 1) * 128],
                                      1.0, diag=True)
            tri2_v = tri2[:].rearrange("p (y q) -> p y q", y=2)
            V_sb = [persist.tile([128, H * W65], bf16, tag=f"V{j}", name=f"V{j}") for j in range(ST)]
            for j in range(ST):
                ones_view = V_sb[j][:].rearrange("p (i w) -> p i w", w=W65)[:, :, DH:W65]
                nc.vector.memset(ones_view, 1.0)

            # ---- persistent E tiles (2 sets), masked prefixes zeroed on DVE ----
            E_sets = []
            for sidx in range(2):
                E_sets.append([
                    persist.tile([128, 2048 if j <= 3 else 1024], bf16,
                                 tag=f"E{sidx}_{j}", name=f"E{sidx}_{j}")
                    for j in range(ST)])
            for sidx in range(2):
                for j in range(ST):
                    w = 128 * (j % 4)
                    if w == 0:
                        continue
                    ybase = 1024 if j <= 3 else 512
                    for y in range(2):
                        nc.vector.memset(E_sets[sidx][j][:, y * ybase:y * ybase + w], 0.0)

            # ---- DMA stream (nc.sync, in arrival-priority order) ----
            # Group-0 W rows get a small dedicated ring so they land by
            # ~9us (sharing the x ring would chain their DMAs behind the
            # x-transpose readers); everything later shares the x ring.
            wq_nm = W_Q.ap().rearrange("i h m -> (i h) m")
            wk_nm = W_K.ap().rearrange("i h m -> (i h) m")
            wv_nm = W_V.ap().rearrange("i h m -> (i h) m")
            wrow = {"q": [None] * MT, "k": [None] * MT, "v": [None] * MT}
            xrow = [None] * ST

            def load_xrows(js):
                for j in js:
                    xr = xstage.tile([128, D], f32, tag="xw", name="xw",
                                     bufs=6 if debug else 8)
                    nc.sync.dma_start(xr[:], x.ap()[j * 128:(j + 1) * 128, :])
                    xrow[j] = xr

            def load_wrows(which, src, rs, tag="xw", bufs=None):
                for r in rs:
                    wr = xstage.tile([128, D], f32, tag=tag, name=tag,
                                     bufs=bufs or (6 if debug else 8))
                    nc.sync.dma_start(wr[:], src[r * 128:(r + 1) * 128, :])
                    wrow[which][r] = wr

            load_xrows(range(0, 4))
            load_wrows("q", wq_nm, range(0, 3), tag="wr6", bufs=5)
            load_wrows("k", wk_nm, range(0, 2), tag="wr6", bufs=5)
            load_wrows("k", wk_nm, range(2, 3))
            load_xrows(range(4, 8))
            load_wrows("v", wv_nm, range(0, 6))
            load_wrows("q", wq_nm, range(3, 6))
            load_wrows("k", wk_nm, range(3, 6))
            # W_O: per-head contiguous [D, DH] f32 loads, last in the sync
            # queue (mid-run, when the DMA engines are otherwise idle); the
            # PE transposes them as pair-1..3 fillers.
            wostage = []
            for i in range(H):
                wo_st = xstage.tile([128, 384], f32, tag="wo", name="wo", bufs=5)
                nc.sync.dma_start(
                    wo_st[:].rearrange("p (a h) -> p a h", h=DH),
                    W_O.ap()[i].rearrange("(a p) h -> p a h", p=128))
                wostage.append(wo_st)

            wqT = [persist.tile([128, D], bf16, tag=f"wqT{m}", name=f"wqT{m}") for m in range(MT)]
            wkT = [persist.tile([128, D], bf16, tag=f"wkT{m}", name=f"wkT{m}") for m in range(MT)]
            wvT = [persist.tile([128, D], bf16, tag=f"wvT{m}", name=f"wvT{m}") for m in range(MT)]
            woT = [persist.tile([128, D], bf16, tag=f"woT{t}", name=f"woT{t}") for t in range(NT)]
            xT = [persist.tile([128, S], bf16, tag=f"xT{m}", name=f"xT{m}") for m in range(MT)]

            evict_flip = [0]

            def evict(dst, src_psum, alt=True):
                # alternate PSUM->SBUF evictions between DVE and ACT; keep
                # them DVE-only (alt=False) where ACT is the busier engine
                if not alt or evict_flip[0] % 2 == 0:
                    nc.vector.tensor_copy(dst, src_psum)
                else:
                    nc.scalar.activation(dst, src_psum,
                                         mybir.ActivationFunctionType.Copy)
                evict_flip[0] += 1

            # Transpose batches write bf16 straight to PSUM (a bank holds
            # 1024 bf16) so the eviction copies are 16-bit (2x DVE mode) and
            # cheap; batches alternate between the ps_mm and the
            # (startup-idle) ps_sc pool for an effective ring depth of 4 so
            # the evictions never pace the PE.
            tp_flip = [0]
            in_startup = [True]   # ps_sc is only free before scores begin

            def tp_tile():
                tp_flip[0] += 1
                if not in_startup[0] or tp_flip[0] % 2 == 0:
                    return ps_mm.tile([128, 512], f32, tag="mm", name="mm")
                return ps_sc.tile([128, 512], f32, tag="sc", name="sc")

            def emit_x_transposes(jg):
                # 4 transposes share one PSUM tile -> one batched evict
                for m in range(MT):
                    if m == 3:
                        sprinkle()
                    pt = tp_tile()
                    for dj in range(4):
                        nc.tensor.transpose(
                            pt[0:128, dj * 128:(dj + 1) * 128],
                            xrow[jg + dj][:, m * 128:(m + 1) * 128],
                            ident[:])
                    evict(xT[m][:, jg * 128:(jg + 4) * 128], pt[:])

            def emit_w_transposes(which, dstT, g):
                for m in range(MT):
                    if m == 3 and in_startup[0]:
                        sprinkle()
                    pt = tp_tile()
                    for dr in range(3):
                        nc.tensor.transpose(
                            pt[0:128, dr * 128:(dr + 1) * 128],
                            wrow[which][3 * g + dr][:, m * 128:(m + 1) * 128],
                            ident[:])
                    evict(dstT[m][:, g * 384:(g + 1) * 384], pt[0:128, 0:384])

            QT = [persist.tile([128, S], bf16, tag=f"QT{t}", name=f"QT{t}") for t in range(NT)]
            KT = [persist.tile([128, S], bf16, tag=f"KT{t}", name=f"KT{t}") for t in range(NT)]
            ZT = [persist.tile([128, S], bf16, tag=f"ZT{t}", name=f"ZT{t}") for t in range(NT)]

            def emit_qkt_chain(t, which, c, alt=True):
                dstT, wT = ((QT, wqT) if which == 0 else (KT, wkT))
                pq = ps_mm.tile([128, 512], f32, tag="mm", name="mm")
                for m in range(MT):
                    nc.tensor.matmul(
                        pq[:],
                        wT[m][:, t * 128:(t + 1) * 128],
                        xT[m][:, c * 512:(c + 1) * 512],
                        start=(m == 0), stop=(m == MT - 1),
                    )
                evict(dstT[t][:, c * 512:(c + 1) * 512], pq[:], alt=alt)

            def emit_v_tile(j):
                for c2 in range(2):  # n chunks of 384
                    pv = ps_mm.tile([128, 512], f32, tag="mm", name="mm")
                    for m in range(MT):
                        nc.tensor.matmul(
                            pv[:, 0:384],
                            xT[m][:, j * 128:(j + 1) * 128],
                            wvT[m][:, c2 * 384:(c2 + 1) * 384],
                            start=(m == 0), stop=(m == MT - 1),
                        )
                    dst = V_sb[j][:].rearrange("p (i w) -> p i w", w=W65)[:, c2 * 6:(c2 + 1) * 6, 0:DH]
                    src = pv[:, 0:384].rearrange("p (i w) -> p i w", w=DH)
                    evict(dst, src)

            def tri_mask2(E_t, j, off):
                # causal mask on both heads' diagonal 128x128 blocks (two
                # contiguous DVE ops — contiguous keeps the 2x 16-bit DVE
                # mode).  MUST stay on DVE: gpsimd swaps Q7 ucode libraries
                # between op families, and each LIBRARY_RELOAD stalls every
                # engine for ~7us.
                ybase = 1024 if j <= 3 else 512
                for y in range(2):
                    dslice = E_t[j][:, y * ybase + off:y * ybase + off + 128]
                    nc.vector.tensor_mul(dslice, dslice, tri2[:, 0:128])

            def emit_score_pair(t, E_t, j):
                # Two heads of the pair in alternating 64-row groups so the
                # PE runs them concurrently per sub-array.  Only causally
                # live 128-col blocks are computed / exponentiated.
                off = 128 * (j % 4)   # masked prefix within the diag chunk
                if j <= 3:
                    scs = []
                    for y in range(2):
                        hb = 64 * y
                        lhsT = KT[t][hb:hb + 64, j * 128:(j + 1) * 128]
                        sc = ps_sc.tile([128, 1024], f32, tag="sc", name="sc")
                        scs.append(sc)
                        nc.tensor.matmul(sc[:, off:512], lhsT,
                                         QT[t][hb:hb + 64, off:512],
                                         start=True, stop=True)
                    for y in range(2):
                        hb = 64 * y
                        lhsT = KT[t][hb:hb + 64, j * 128:(j + 1) * 128]
                        nc.tensor.matmul(scs[y][:, 512:1024], lhsT,
                                         QT[t][hb:hb + 64, 512:1024],
                                         start=True, stop=True)
                    for y in range(2):
                        nc.scalar.activation(
                            E_t[j][:, y * 1024 + off:y * 1024 + 1024],
                            scs[y][:, off:1024],
                            mybir.ActivationFunctionType.Exp, scale=SCALE)
                    tri_mask2(E_t, j, off)
                else:
                    # one PSUM tile holds both heads (one bank each) and a
                    # single strided exp covers them: halves the ACT
                    # per-instruction overhead for the j>=4 half
                    sc = ps_sc.tile([128, 1024], f32, tag="sc", name="sc")
                    for y in range(2):
                        hb = 64 * y
                        lhsT = KT[t][hb:hb + 64, j * 128:(j + 1) * 128]
                        nc.tensor.matmul(sc[:, y * 512 + off:(y + 1) * 512],
                                         lhsT,
                                         QT[t][hb:hb + 64, 512 + off:1024],
                                         start=True, stop=True)
                    src = sc[:].rearrange("p (y q) -> p y q", y=2)[:, :, off:512]
                    dstv = E_t[j][:].rearrange("p (y q) -> p y q", y=2)[:, :, off:512]
                    nc.scalar.activation(dstv, src,
                                         mybir.ActivationFunctionType.Exp,
                                         scale=SCALE)
                    tri_mask2(E_t, j, off)

            def emit_z_chain_q256(t, E_t, k, y):
                # 256-col sub-chunk of c1 (queries [512+256k, 768+256k)):
                # lets the last pair's out-proj rows start before all of c1
                # is done.  Key blocks above the chunk's top query are
                # skipped; masked sub-ranges are zeros in E.
                qlo = 512 + 256 * k
                jmax = 5 if k == 0 else 7
                i = 2 * t + y
                zt = ps_zt.tile([128, 512], f32, tag="zt", name="zt")
                for j in range(jmax + 1):
                    if j <= 3:
                        rhs = E_t[j][:, y * 1024 + qlo: y * 1024 + qlo + 256]
                    else:
                        rhs = E_t[j][:, y * 512 + 256 * k: y * 512 + 256 * k + 256]
                    nc.tensor.matmul(
                        zt[0:65, 0:256],
                        V_sb[j][:, i * W65:(i + 1) * W65],
                        rhs,
                        start=(j == 0), stop=(j == jmax),
                    )
                den = small.tile([1, 512], f32, tag="den", name="den")
                nc.vector.tensor_copy(den[:, 0:256], zt[64:65, 0:256])
                nc.vector.reciprocal_approx_fast(den[:, 0:256], den[:, 0:256])
                bc = small.tile([64, 512], f32, tag="bc", name="bc")
                nc.gpsimd.partition_broadcast(bc[:, 0:256], den[:, 0:256])
                nc.vector.tensor_mul(
                    ZT[t][64 * y:64 * y + 64, qlo:qlo + 256],
                    zt[0:64, 0:256], bc[:, 0:256])

            def emit_z_chain(t, E_t, c, y):
                jmax = 4 * c + 3
                i = 2 * t + y
                zt = ps_zt.tile([128, 512], f32, tag="zt", name="zt")
                for j in range(jmax + 1):
                    if j <= 3:
                        rhs = E_t[j][:, y * 1024 + c * 512: y * 1024 + (c + 1) * 512]
                    else:
                        rhs = E_t[j][:, y * 512:(y + 1) * 512]
                    nc.tensor.matmul(
                        zt[0:65, :],
                        V_sb[j][:, i * W65:(i + 1) * W65],
                        rhs,
                        start=(j == 0), stop=(j == jmax),
                    )
                den = small.tile([1, 512], f32, tag="den", name="den")
                nc.vector.tensor_copy(den[:], zt[64:65, :])
                nc.vector.reciprocal_approx_fast(den[:], den[:])
                bc = small.tile([64, 512], f32, tag="bc", name="bc")
                nc.gpsimd.partition_broadcast(bc[:], den[:])
                nc.vector.tensor_mul(
                    ZT[t][64 * y:64 * y + 64, c * 512:(c + 1) * 512],
                    zt[0:64, :], bc[:])

            def emit_outproj_row(qj, tail=False, chunked=False, dve_evict=False):
                # In the tail (after the last scores) the sc pool is free —
                # alternate PSUM pools for ring depth 4, and evict via the
                # then-idle ACT so the DVE's z-norm queue can't pace the PE.
                # One full-row DMA (vs per-384-chunk) halves the sync-queue
                # dispatches that drain after the last matmul.
                osb = outsb_pool.tile([128, D], f32, tag="osb", name="osb")
                for mc in range(2):
                    if tail and mc == 1:
                        po = ps_sc.tile([128, 512], f32, tag="sc", name="sc")
                    else:
                        po = ps_mm.tile([128, 512], f32, tag="mm", name="mm")
                    for t in range(NT):
                        nc.tensor.matmul(
                            po[:, 0:384],
                            ZT[t][:, qj * 128:(qj + 1) * 128],
                            woT[t][:, mc * 384:(mc + 1) * 384],
                            start=(t == 0), stop=(t == NT - 1),
                        )
                    if tail:
                        nc.scalar.activation(osb[:, mc * 384:(mc + 1) * 384],
                                             po[:, 0:384],
                                             mybir.ActivationFunctionType.Copy)
                    elif dve_evict:
                        # rows 0-1 run while pair-5 exps are still pending:
                        # an ACT Copy here would delay exp(j7) and stall the
                        # c1b z-chains behind it
                        nc.vector.tensor_copy(osb[:, mc * 384:(mc + 1) * 384],
                                              po[:, 0:384])
                    else:
                        evict(osb[:, mc * 384:(mc + 1) * 384], po[:, 0:384])
                    if chunked:
                        nc.sync.dma_start(
                            out.ap()[qj * 128:(qj + 1) * 128,
                                     mc * 384:(mc + 1) * 384],
                            osb[:, mc * 384:(mc + 1) * 384])
                if not chunked:
                    nc.sync.dma_start(out.ap()[qj * 128:(qj + 1) * 128, :], osb[:])

            # ---- schedule ----
            # HAM warm-up: dummy matmuls while the x rows stream in (PE
            # transpose-mode ops don't count as PE-busy for the HAM clock
            # gate).  A short ident-based f32 chain starts as soon as gpsimd
            # finishes make_identity; short "sprinkle" matmuls between the
            # DMA-gated transpose batches keep the clock warm through the
            # whole startup phase.
            warm_ps = ps_zt.tile([128, 512], f32, tag="zt", name="zt")
            for w in range(4):
                nc.tensor.matmul(warm_ps[:, 0:128], ident[:], ident[:],
                                 start=True, stop=True)
            NWARM = 10
            for w in range(NWARM):
                nc.tensor.matmul(warm_ps[:], warm_src[:, 0:128], warm_src[:],
                                 start=(w == 0), stop=(w == NWARM - 1))

            def sprinkle():
                nc.tensor.matmul(warm_ps[:, 0:256], warm_src[:, 0:128],
                                 warm_src[:, 0:256], start=True, stop=True)

            emit_x_transposes(0)
            sprinkle()
            emit_w_transposes("q", wqT, 0)
            sprinkle()
            emit_qkt_chain(0, 0, 0)
            emit_x_transposes(4)
            sprinkle()
            emit_w_transposes("k", wkT, 0)
            sprinkle()
            emit_qkt_chain(0, 0, 1)
            emit_qkt_chain(0, 1, 0)
            sprinkle()
            warm_out = small.tile([1, 512], f32, tag="den", name="den")
            nc.vector.tensor_copy(warm_out[:], warm_ps[0:1, :])
            nc.sync.dma_start(warmout.ap(), warm_out[:])

            def emit_wo_transpose_group(i):
                # wostage[i] is W_O[i] as [128 m-rows, 6 a-blocks x 64 h];
                # transpose the six [128,64] blocks -> woT[i//2] row half.
                t, hb = i // 2, 64 * (i % 2)
                for g in range(2):
                    pt = ps_mm.tile([128, 512], f32, tag="mm", name="mm")
                    for a in range(3):
                        nc.tensor.transpose(
                            pt[0:64, a * 128:(a + 1) * 128],
                            wostage[i][:, (3 * g + a) * 64:(3 * g + a + 1) * 64],
                            ident[:])
                    evict(woT[t][hb:hb + 64, g * 384:(g + 1) * 384],
                          pt[0:64, 0:384])

            # ---- pair 0 (special: V projection + late z) ----
            # Filler order tracks DMA arrivals: QKT(1) first (its weights are
            # already transposed), W_V transposes last (rows land latest).
            in_startup[0] = False   # ps_sc now belongs to the score tiles
            E0 = E_sets[0]
            emit_score_pair(0, E0, 0)
            emit_score_pair(0, E0, 1)
            emit_qkt_chain(0, 1, 1)
            sprinkle()
            p0_fillers = [lambda w=w, c=c: emit_qkt_chain(1, w, c)
                          for w in range(2) for c in range(PC)]
            p0_fillers += [sprinkle, lambda: emit_w_transposes("v", wvT, 0)]
            fi = 0
            for j in range(2, ST):
                emit_score_pair(0, E0, j)
                if fi < len(p0_fillers):
                    p0_fillers[fi]()
                    fi += 1
            while fi < len(p0_fillers):
                p0_fillers[fi]()
                fi += 1
            emit_w_transposes("v", wvT, 1)
            for j in range(4):
                emit_v_tile(j)
            emit_z_chain(0, E0, 0, 0)
            emit_z_chain(0, E0, 0, 1)
            emit_score_pair(1, E_sets[1], 0)
            emit_score_pair(1, E_sets[1], 1)
            for j in range(4, ST):
                emit_v_tile(j)
            emit_score_pair(1, E_sets[1], 2)
            emit_score_pair(1, E_sets[1], 3)
            emit_z_chain(0, E0, 1, 0)
            emit_z_chain(0, E0, 1, 1)

            # ---- pairs 1..5 ----
            for t in range(1, NT):
                E_t = E_sets[t % 2]
                if t == 1:
                    # group-1 weight rows land last: transpose them here
                    fillers = [lambda: emit_w_transposes("q", wqT, 1),
                               lambda: emit_w_transposes("k", wkT, 1)]
                    fillers += [lambda w=w, c=c: emit_qkt_chain(2, w, c)
                                for w in range(2) for c in range(PC)]
                elif t + 1 < NT:
                    fillers = [lambda w=w, c=c, tt=t + 1: emit_qkt_chain(tt, w, c)
                               for w in range(2) for c in range(PC)]
                else:
                    fillers = []   # rows start after the loop: any evict
                                   # emitted before exp(j7) delays the
                                   # tri(j7) -> c1b dependency chain
                if t in (1, 2, 3):
                    fillers += [lambda i=i: emit_wo_transpose_group(i)
                                for i in range(4 * (t - 1), 4 * t)]
                fi = 0
                for j in range(4, ST):
                    emit_score_pair(t, E_t, j)
                    if j == 4:
                        emit_z_chain(t, E_t, 0, 0)
                    elif j == 5:
                        emit_z_chain(t, E_t, 0, 1)
                        if t + 1 == NT:
                            # last pair: first 256-col half of c1 as soon as
                            # its key blocks (j<=5) are exponentiated
                            emit_z_chain_q256(t, E_t, 0, 0)
                    elif t + 1 == NT and j == 6:
                        emit_z_chain_q256(t, E_t, 0, 1)
                    # out-proj fillers need z(c0) of pair 5: only after j==5
                    if (t + 1 < NT or j >= 6) and fi < len(fillers):
                        fillers[fi]()
                        fi += 1
                while fi < len(fillers):
                    fillers[fi]()
                    fi += 1
                # prefetch ALL of the next pair's j<=3 score tiles,
                # interleaved with this pair's c1 chains: the next pair's
                # tri-masks then precede these norms in the DVE queue, so
                # its z(c0) chains start unstalled at j==4
                if t + 1 < NT:
                    E_next = E_sets[(t + 1) % 2]
                    emit_score_pair(t + 1, E_next, 0)
                    emit_score_pair(t + 1, E_next, 1)
                    emit_z_chain(t, E_t, 1, 0)
                    emit_score_pair(t + 1, E_next, 2)
                    emit_score_pair(t + 1, E_next, 3)
                    emit_z_chain(t, E_t, 1, 1)
                else:
                    # last pair: c1b chains immediately (their exps are
                    # done), rows 2-3 give the PE work while the c1a norms
                    # drain, then rows 4-7 land as their ZT columns finish
                    emit_outproj_row(0, tail=True)
                    emit_outproj_row(1, tail=True)
                    emit_outproj_row(2, tail=True)
                    emit_outproj_row(3, tail=True)
                    emit_z_chain_q256(t, E_t, 1, 0)
                    emit_z_chain_q256(t, E_t, 1, 1)
                    for qj in range(4, ST):
                        emit_outproj_row(qj, tail=True, chunked=(qj >= 6))

            if debug:
                dpool = ctx.enter_context(tc.tile_pool(name="dpool", bufs=1))

                def dump(name, tile_ap):
                    fs = 1
                    for s_ in tile_ap.shape[1:]:
                        fs *= s_
                    f = dpool.tile([128, fs], f32, tag="d", name="d")
                    nc.vector.tensor_copy(f[:, 0:fs], tile_ap)
                    nc.sync.dma_start(dbg[name].ap(), f[:, 0:fs])

                for t in range(NT):
                    dump(f"dZT{t}", ZT[t][:])
                    dump(f"dQT{t}", QT[t][:])
                    dump(f"dKT{t}", KT[t][:])
                    dump(f"dwoT{t}", woT[t][:])
                for j in range(ST):
                    dump(f"dV{j}", V_sb[j][:])
                    # E set 1 holds pair 5's post-exp/mask values at the end
                    nc.sync.dma_start(dbg[f"dE{j}"].ap(), E_sets[1][j][:])

    nc.compile()
    return nc


_NC_CACHE = None


def _get_nc():
    global _NC_CACHE
    if _NC_CACHE is None:
        _NC_CACHE = build(N_CORES)
    return _NC_CACHE


def run(inputs, trace=False, **kwargs):
    nc = _get_nc()
    weights = {k: np.ascontiguousarray(np.asarray(inputs[k], dtype=np.float32))
               for k in ("W_Q", "W_K", "W_V", "W_O")}
    xs = np.ascontiguousarray(np.asarray(inputs["x"], dtype=np.float32))
    in_maps = [dict(weights, x=xs[b]) for b in range(B)]
    res = run_bass_kernel_spmd(nc, in_maps, core_ids=list(range(N_CORES)),
                               trace=trace, **kwargs)
    out = np.stack([np.asarray(res.results[b]["out"]) for b in range(B)], axis=0)
    return out.astype(np.float32), res


def kernel(**inputs) -> np.ndarray:
    out, _ = run(inputs, trace=False)
    return out


# revision 65
# speedup vs baseline: 1.1794x; 1.1794x over previous
"""Distributed causal multi-head attention for Trainium2 (8 NeuronCores).

Problem: B=8, S=1024, D=768, H=12, DH=64 causal MHA (dense_transformer).
Sharding: pure data parallel — batch element b runs on core b; weights are
replicated. No collectives.

Per-core kernel (bf16 TensorE compute, f32 PSUM accumulation):
  1. Startup is HBM-stream-bound, so only the tensors that gate compute are
     loaded up front, in arrival order: x rows, then W_Q/W_K rows 0-2, all
     W_V rows, W_Q/W_K rows 3-5 — all f32 via HWDGE on one queue, transposed
     on the PE (streamed row-group by row-group, interleaved with the first
     QKT chains so head-pair work starts as each row lands).  W_O loads
     mid-run (12 contiguous per-head [D,DH] chunks, last in the sync queue)
     and is PE-transposed as pair-1..3 fillers.  HAM clock warm-up: an
     ident-chain + short sprinkle matmuls bridge the DMA-gated stretches
     (PE-transpose-mode ops don't count as PE-busy for the clock gate).
  2. QKV projections on TensorE -> QT/KT [n,s] (transposed) and V [s,n],
     PSUM evictions alternating between VectorE and ScalarE, transpose
     batches alternating between two PSUM pools (ring depth 4) so
     evictions never pace the PE.
  3. Scores computed transposed per head: SC[q,p] = sum_h K[q,h] Q[p,h],
     two heads in alternating 64-row groups (concurrent PE sub-arrays),
     skipping causally-dead 128-col blocks; exp(SC/8) on ScalarE covers
     only causally-live columns (one merged 2-head exp for j>=4).  Masked
     column prefixes of the persistent double-buffered E tiles are zeroed
     once at startup; the diagonal 128x128 block is masked by multiplying
     with an upper-triangular tile on DVE (NEVER gpsimd: mixing op families
     there forces ~7us Q7 LIBRARY_RELOAD stalls).
  4. z^T = V^T E accumulated on TensorE with a ones-column per head riding
     the same matmul to produce softmax denominators; normalize with
     copy->reciprocal_approx_fast (PSUM-direct custom-DVE reads are
     broken) + gpsimd partition_broadcast (source must sit on partition 0)
     + VectorE multiply.
  5. Output projection from zT/woT tiles: rows 0-1 run as PE filler inside
     head-pair 5 (they only need ZT[:, 0:512]); pair 5's c1 is split into
     two 256-col z chains so rows 2-7 start as their ZT columns finish;
     tail evictions go to the then-idle ScalarE and one full-row DMA per
     128-row block minimizes the final sync-queue drain.
"""
import numpy as np

import concourse.bacc as bacc
import concourse.mybir as mybir
import concourse.tile as tile
from concourse.masks import make_identity, make_upper_triangular
from concourse.bass_utils import run_bass_kernel_spmd

f32 = mybir.dt.float32
bf16 = mybir.dt.bfloat16

B = 8
S, D, H, DH = 1024, 768, 12, 64
NT = 6    # n 128-tiles (head pairs)
MT = 6    # m 128-tiles
ST = 8    # s 128-tiles
PC = 2    # p chunks of 512
SCALE = 0.125  # 1/sqrt(DH)
W65 = DH + 1   # per-head V columns incl the ones column

N_CORES = 8


def build(n_cores: int = N_CORES, debug: bool = False):
    nc = bacc.Bacc("TRN2", target_bir_lowering=False, debug=False, num_devices=n_cores,
                   num_swdge_queues=4)

    x = nc.dram_tensor("x", [S, D], f32, kind="ExternalInput")
    W_Q = nc.dram_tensor("W_Q", [H, DH, D], f32, kind="ExternalInput")
    W_K = nc.dram_tensor("W_K", [H, DH, D], f32, kind="ExternalInput")
    W_V = nc.dram_tensor("W_V", [H, DH, D], f32, kind="ExternalInput")
    W_O = nc.dram_tensor("W_O", [H, D, DH], f32, kind="ExternalInput")
    out = nc.dram_tensor("out", [S, D], f32, kind="ExternalOutput")

    warmout = nc.dram_tensor("warmout", [1, 512], f32)

    dbg = {}
    if debug:
        for t in range(NT):
            dbg[f"dZT{t}"] = nc.dram_tensor(f"dZT{t}", [128, S], f32, kind="ExternalOutput")
            dbg[f"dQT{t}"] = nc.dram_tensor(f"dQT{t}", [128, S], f32, kind="ExternalOutput")
            dbg[f"dKT{t}"] = nc.dram_tensor(f"dKT{t}", [128, S], f32, kind="ExternalOutput")
            dbg[f"dwoT{t}"] = nc.dram_tensor(f"dwoT{t}", [128, D], f32, kind="ExternalOutput")
        for j in range(ST):
            dbg[f"dV{j}"] = nc.dram_tensor(f"dV{j}", [128, H * W65], f32, kind="ExternalOutput")
            dbg[f"dE{j}"] = nc.dram_tensor(f"dE{j}", [128, 2048 if j <= 3 else 1024], bf16,
                                           kind="ExternalOutput")

    with tile.TileContext(nc) as tc:
        from contextlib import ExitStack
        with ExitStack() as ctx:
            persist = ctx.enter_context(tc.tile_pool(name="persist", bufs=1))
            xstage = ctx.enter_context(tc.tile_pool(name="xstage", bufs=8))
            outsb_pool = ctx.enter_context(tc.tile_pool(name="outsb", bufs=2))
            small = ctx.enter_context(tc.tile_pool(name="small", bufs=2))
            ps_mm = ctx.enter_context(tc.tile_pool(name="ps_mm", bufs=2, space="PSUM"))
            ps_sc = ctx.enter_context(tc.tile_pool(name="ps_sc", bufs=2, space="PSUM"))
            ps_zt = ctx.enter_context(tc.tile_pool(name="ps_zt", bufs=2, space="PSUM"))

            # ---- init: DVE handles memsets (keeps the gpsimd queue nearly
            # empty — only ident/tri build + the z broadcasts live there) ----
            warm_src = persist.tile([128, 512], bf16, tag="warm", name="warm")
            nc.vector.memset(warm_src[:], 1.0)
            ident = persist.tile([128, 128], f32, tag="ident", name="ident")
            make_identity(nc, ident[:])
            tri2 = persist.tile([128, 256], bf16, tag="tri", name="tri")
            for half in range(2):
                make_upper_triangular(nc, tri2[:, half * 128:(half + 1) * 128],
                                      1.0, diag=True)
            tri2_v = tri2[:].rearrange("p (y q) -> p y q", y=2)
            V_sb = [persist.tile([128, H * W65], bf16, tag=f"V{j}", name=f"V{j}") for j in range(ST)]
            for j in range(ST):
                ones_view = V_sb[j][:].rearrange("p (i w) -> p i w", w=W65)[:, :, DH:W65]
                nc.vector.memset(ones_view, 1.0)

            # ---- persistent E tiles (2 sets), masked prefixes zeroed on DVE ----
            E_sets = []
            for sidx in range(2):
                E_sets.append([
                    persist.tile([128, 2048 if j <= 3 else 1024], bf16,
                                 tag=f"E{sidx}_{j}", name=f"E{sidx}_{j}")
                    for j in range(ST)])
            for sidx in range(2):
                for j in range(ST):
                    w = 128 * (j % 4)
                    if w == 0:
                        continue
                    ybase = 1024 if j <= 3 else 512
                    for y in range(2):
                        nc.vector.memset(E_sets[sidx][j][:, y * ybase:y * ybase + w], 0.0)

            # ---- DMA stream (nc.sync, in arrival-priority order) ----
            # Group-0 W rows get a small dedicated ring so they land by
            # ~9us (sharing the x ring would chain their DMAs behind the
            # x-transpose readers); everything later shares the x ring.
            wq_nm = W_Q.ap().rearrange("i h m -> (i h) m")
            wk_nm = W_K.ap().rearrange("i h m -> (i h) m")
            wv_nm = W_V.ap().rearrange("i h m -> (i h) m")
            wrow = {"q": [None] * MT, "k": [None] * MT, "v": [None] * MT}
            xrow = [None] * ST

            def load_xrows(js):
                for j in js:
                    xr = xstage.tile([128, D], f32, tag="xw", name="xw",
                                     bufs=6 if debug else 8)
                    nc.sync.dma_start(xr[:], x.ap()[j * 128:(j + 1) * 128, :])
                    xrow[j] = xr

            def load_wrows(which, src, rs, tag="xw", bufs=None):
                for r in rs:
                    wr = xstage.tile([128, D], f32, tag=tag, name=tag,
                                     bufs=bufs or (6 if debug else 8))
                    nc.sync.dma_start(wr[:], src[r * 128:(r + 1) * 128, :])
                    wrow[which][r] = wr

            load_xrows(range(0, 4))
            load_wrows("q", wq_nm, range(0, 3), tag="wr6", bufs=5)
            load_wrows("k", wk_nm, range(0, 2), tag="wr6", bufs=5)
            load_wrows("k", wk_nm, range(2, 3))
            load_xrows(range(4, 8))
            load_wrows("v", wv_nm, range(0, 6))
            load_wrows("q", wq_nm, range(3, 6))
            load_wrows("k", wk_nm, range(3, 6))
            # W_O: per-head contiguous [D, DH] f32 loads, last in the sync
            # queue (mid-run, when the DMA engines are otherwise idle); the
            # PE transposes them as pair-1..3 fillers.
            wostage = []
            for i in range(H):
                wo_st = xstage.tile([128, 384], f32, tag="wo", name="wo", bufs=4)
                nc.sync.dma_start(
                    wo_st[:].rearrange("p (a h) -> p a h", h=DH),
                    W_O.ap()[i].rearrange("(a p) h -> p a h", p=128))
                wostage.append(wo_st)

            wqT = [persist.tile([128, D], bf16, tag=f"wqT{m}", name=f"wqT{m}") for m in range(MT)]
            wkT = [persist.tile([128, D], bf16, tag=f"wkT{m}", name=f"wkT{m}") for m in range(MT)]
            wvT = [persist.tile([128, D], bf16, tag=f"wvT{m}", name=f"wvT{m}") for m in range(MT)]
            woT = [persist.tile([128, D], bf16, tag=f"woT{t}", name=f"woT{t}") for t in range(NT)]
            xT = [persist.tile([128, S], bf16, tag=f"xT{m}", name=f"xT{m}") for m in range(MT)]

            evict_flip = [0]

            def evict(dst, src_psum, alt=True):
                # alternate PSUM->SBUF evictions between DVE and ACT; keep
                # them DVE-only (alt=False) where ACT is the busier engine
                if not alt or evict_flip[0] % 2 == 0:
                    nc.vector.tensor_copy(dst, src_psum)
                else:
                    nc.scalar.activation(dst, src_psum,
                                         mybir.ActivationFunctionType.Copy)
                evict_flip[0] += 1

            # Transpose batches write bf16 straight to PSUM (a bank holds
            # 1024 bf16) so the eviction copies are 16-bit (2x DVE mode) and
            # cheap; batches alternate between the ps_mm and the
            # (startup-idle) ps_sc pool for an effective ring depth of 4 so
            # the evictions never pace the PE.
            tp_flip = [0]
            in_startup = [True]   # ps_sc is only free before scores begin

            def tp_tile():
                tp_flip[0] += 1
                if not in_startup[0] or tp_flip[0] % 2 == 0:
                    return ps_mm.tile([128, 512], f32, tag="mm", name="mm")
                return ps_sc.tile([128, 512], f32, tag="sc", name="sc")

            def emit_x_transposes(jg):
                # 4 transposes share one PSUM tile -> one batched evict
                for m in range(MT):
                    if m == 3:
                        sprinkle()
                    pt = tp_tile()
                    for dj in range(4):
                        nc.tensor.transpose(
                            pt[0:128, dj * 128:(dj + 1) * 128],
                            xrow[jg + dj][:, m * 128:(m + 1) * 128],
                            ident[:])
                    evict(xT[m][:, jg * 128:(jg + 4) * 128], pt[:])

            def emit_w_transposes(which, dstT, g):
                for m in range(MT):
                    if m == 3 and in_startup[0]:
                        sprinkle()
                    pt = tp_tile()
                    for dr in range(3):
                        nc.tensor.transpose(
                            pt[0:128, dr * 128:(dr + 1) * 128],
                            wrow[which][3 * g + dr][:, m * 128:(m + 1) * 128],
                            ident[:])
                    evict(dstT[m][:, g * 384:(g + 1) * 384], pt[0:128, 0:384])

            QT = [persist.tile([128, S], bf16, tag=f"QT{t}", name=f"QT{t}") for t in range(NT)]
            KT = [persist.tile([128, S], bf16, tag=f"KT{t}", name=f"KT{t}") for t in range(NT)]
            ZT = [persist.tile([128, S], bf16, tag=f"ZT{t}", name=f"ZT{t}") for t in range(NT)]

            def emit_qkt_chain(t, which, c, alt=True):
                dstT, wT = ((QT, wqT) if which == 0 else (KT, wkT))
                pq = ps_mm.tile([128, 512], f32, tag="mm", name="mm")
                for m in range(MT):
                    nc.tensor.matmul(
                        pq[:],
                        wT[m][:, t * 128:(t + 1) * 128],
                        xT[m][:, c * 512:(c + 1) * 512],
                        start=(m == 0), stop=(m == MT - 1),
                    )
                evict(dstT[t][:, c * 512:(c + 1) * 512], pq[:], alt=alt)

            def emit_v_tile(j):
                for c2 in range(2):  # n chunks of 384
                    pv = ps_mm.tile([128, 512], f32, tag="mm", name="mm")
                    for m in range(MT):
                        nc.tensor.matmul(
                            pv[:, 0:384],
                            xT[m][:, j * 128:(j + 1) * 128],
                            wvT[m][:, c2 * 384:(c2 + 1) * 384],
                            start=(m == 0), stop=(m == MT - 1),
                        )
                    dst = V_sb[j][:].rearrange("p (i w) -> p i w", w=W65)[:, c2 * 6:(c2 + 1) * 6, 0:DH]
                    src = pv[:, 0:384].rearrange("p (i w) -> p i w", w=DH)
                    evict(dst, src)

            def tri_mask2(E_t, j, off):
                # causal mask on both heads' diagonal 128x128 blocks (two
                # contiguous DVE ops — contiguous keeps the 2x 16-bit DVE
                # mode).  MUST stay on DVE: gpsimd swaps Q7 ucode libraries
                # between op families, and each LIBRARY_RELOAD stalls every
                # engine for ~7us.
                ybase = 1024 if j <= 3 else 512
                for y in range(2):
                    dslice = E_t[j][:, y * ybase + off:y * ybase + off + 128]
                    nc.vector.tensor_mul(dslice, dslice, tri2[:, 0:128])

            def emit_score_pair(t, E_t, j):
                # Two heads of the pair in alternating 64-row groups so the
                # PE runs them concurrently per sub-array.  Only causally
                # live 128-col blocks are computed / exponentiated.
                off = 128 * (j % 4)   # masked prefix within the diag chunk
                if j <= 3:
                    scs = []
                    for y in range(2):
                        hb = 64 * y
                        lhsT = KT[t][hb:hb + 64, j * 128:(j + 1) * 128]
                        sc = ps_sc.tile([128, 1024], f32, tag="sc", name="sc")
                        scs.append(sc)
                        nc.tensor.matmul(sc[:, off:512], lhsT,
                                         QT[t][hb:hb + 64, off:512],
                                         start=True, stop=True)
                    for y in range(2):
                        hb = 64 * y
                        lhsT = KT[t][hb:hb + 64, j * 128:(j + 1) * 128]
                        nc.tensor.matmul(scs[y][:, 512:1024], lhsT,
                                         QT[t][hb:hb + 64, 512:1024],
                                         start=True, stop=True)
                    for y in range(2):
                        nc.scalar.activation(
                            E_t[j][:, y * 1024 + off:y * 1024 + 1024],
                            scs[y][:, off:1024],
                            mybir.ActivationFunctionType.Exp, scale=SCALE)
                    tri_mask2(E_t, j, off)
                else:
                    # one PSUM tile holds both heads (one bank each) and a
                    # single strided exp covers them: halves the ACT
                    # per-instruction overhead for the j>=4 half
                    sc = ps_sc.tile([128, 1024], f32, tag="sc", name="sc")
                    for y in range(2):
                        hb = 64 * y
                        lhsT = KT[t][hb:hb + 64, j * 128:(j + 1) * 128]
                        nc.tensor.matmul(sc[:, y * 512 + off:(y + 1) * 512],
                                         lhsT,
                                         QT[t][hb:hb + 64, 512 + off:1024],
                                         start=True, stop=True)
                    src = sc[:].rearrange("p (y q) -> p y q", y=2)[:, :, off:512]
                    dstv = E_t[j][:].rearrange("p (y q) -> p y q", y=2)[:, :, off:512]
                    nc.scalar.activation(dstv, src,
                                         mybir.ActivationFunctionType.Exp,
                                         scale=SCALE)
                    tri_mask2(E_t, j, off)

            def emit_z_chain_q256(t, E_t, k, y):
                # 256-col sub-chunk of c1 (queries [512+256k, 768+256k)):
                # lets the last pair's out-proj rows start before all of c1
                # is done.  Key blocks above the chunk's top query are
                # skipped; masked sub-ranges are zeros in E.
                qlo = 512 + 256 * k
                jmax = 5 if k == 0 else 7
                i = 2 * t + y
                zt = ps_zt.tile([128, 512], f32, tag="zt", name="zt")
                for j in range(jmax + 1):
                    if j <= 3:
                        rhs = E_t[j][:, y * 1024 + qlo: y * 1024 + qlo + 256]
                    else:
                        rhs = E_t[j][:, y * 512 + 256 * k: y * 512 + 256 * k + 256]
                    nc.tensor.matmul(
                        zt[0:65, 0:256],
                        V_sb[j][:, i * W65:(i + 1) * W65],
                        rhs,
                        start=(j == 0), stop=(j == jmax),
                    )
                den = small.tile([1, 512], f32, tag="den", name="den")
                nc.vector.tensor_copy(den[:, 0:256], zt[64:65, 0:256])
                nc.vector.reciprocal_approx_fast(den[:, 0:256], den[:, 0:256])
                bc = small.tile([64, 512], f32, tag="bc", name="bc")
                nc.gpsimd.partition_broadcast(bc[:, 0:256], den[:, 0:256])
                nc.vector.tensor_mul(
                    ZT[t][64 * y:64 * y + 64, qlo:qlo + 256],
                    zt[0:64, 0:256], bc[:, 0:256])

            def emit_z_chain(t, E_t, c, y):
                jmax = 4 * c + 3
                i = 2 * t + y
                zt = ps_zt.tile([128, 512], f32, tag="zt", name="zt")
                for j in range(jmax + 1):
                    if j <= 3:
                        rhs = E_t[j][:, y * 1024 + c * 512: y * 1024 + (c + 1) * 512]
                    else:
                        rhs = E_t[j][:, y * 512:(y + 1) * 512]
                    nc.tensor.matmul(
                        zt[0:65, :],
                        V_sb[j][:, i * W65:(i + 1) * W65],
                        rhs,
                        start=(j == 0), stop=(j == jmax),
                    )
                den = small.tile([1, 512], f32, tag="den", name="den")
                nc.vector.tensor_copy(den[:], zt[64:65, :])
                nc.vector.reciprocal_approx_fast(den[:], den[:])
                bc = small.tile([64, 512], f32, tag="bc", name="bc")
                nc.gpsimd.partition_broadcast(bc[:], den[:])
                nc.vector.tensor_mul(
                    ZT[t][64 * y:64 * y + 64, c * 512:(c + 1) * 512],
                    zt[0:64, :], bc[:])

            def emit_outproj_row(qj, tail=False, chunked=False, dve_evict=False):
                # In the tail (after the last scores) the sc pool is free —
                # alternate PSUM pools for ring depth 4, and evict via the
                # then-idle ACT so the DVE's z-norm queue can't pace the PE.
                # One full-row DMA (vs per-384-chunk) halves the sync-queue
                # dispatches that drain after the last matmul.
                osb = outsb_pool.tile([128, D], f32, tag="osb", name="osb")
                for mc in range(2):
                    if tail and mc == 1:
                        po = ps_sc.tile([128, 512], f32, tag="sc", name="sc")
                    else:
                        po = ps_mm.tile([128, 512], f32, tag="mm", name="mm")
                    for t in range(NT):
                        nc.tensor.matmul(
                            po[:, 0:384],
                            ZT[t][:, qj * 128:(qj + 1) * 128],
                            woT[t][:, mc * 384:(mc + 1) * 384],
                            start=(t == 0), stop=(t == NT - 1),
                        )
                    if tail:
                        nc.scalar.activation(osb[:, mc * 384:(mc + 1) * 384],
                                             po[:, 0:384],
                                             mybir.ActivationFunctionType.Copy)
                    elif dve_evict:
                        # rows 0-1 run while pair-5 exps are still pending:
                        # an ACT Copy here would delay exp(j7) and stall the
                        # c1b z-chains behind it
                        nc.vector.tensor_copy(osb[:, mc * 384:(mc + 1) * 384],
                                              po[:, 0:384])
                    else:
                        evict(osb[:, mc * 384:(mc + 1) * 384], po[:, 0:384])
                    if chunked:
                        nc.sync.dma_start(
                            out.ap()[qj * 128:(qj + 1) * 128,
                                     mc * 384:(mc + 1) * 384],
                            osb[:, mc * 384:(mc + 1) * 384])
                if not chunked:
                    nc.sync.dma_start(out.ap()[qj * 128:(qj + 1) * 128, :], osb[:])

            # ---- schedule ----
            # HAM warm-up: dummy matmuls while the x rows stream in (PE
            # transpose-mode ops don't count as PE-busy for the HAM clock
            # gate).  A short ident-based f32 chain starts as soon as gpsimd
            # finishes make_identity; short "sprinkle" matmuls between the
            # DMA-gated transpose batches keep the clock warm through the
            # whole startup phase.
            warm_ps = ps_zt.tile([128, 512], f32, tag="zt", name="zt")
            for w in range(4):
                nc.tensor.matmul(warm_ps[:, 0:128], ident[:], ident[:],
                                 start=True, stop=True)
            NWARM = 10
            for w in range(NWARM):
                nc.tensor.matmul(warm_ps[:], warm_src[:, 0:128], warm_src[:],
                                 start=(w == 0), stop=(w == NWARM - 1))

            def sprinkle():
                nc.tensor.matmul(warm_ps[:, 0:256], warm_src[:, 0:128],
                                 warm_src[:, 0:256], start=True, stop=True)

            emit_x_transposes(0)
            sprinkle()
            emit_w_transposes("q", wqT, 0)
            sprinkle()
            emit_qkt_chain(0, 0, 0)
            emit_x_transposes(4)
            sprinkle()
            emit_w_transposes("k", wkT, 0)
            sprinkle()
            emit_qkt_chain(0, 0, 1)
            emit_qkt_chain(0, 1, 0)
            sprinkle()
            warm_out = small.tile([1, 512], f32, tag="den", name="den")
            nc.vector.tensor_copy(warm_out[:], warm_ps[0:1, :])
            nc.sync.dma_start(warmout.ap(), warm_out[:])

            def emit_wo_transpose_group(i):
                # wostage[i] is W_O[i] as [128 m-rows, 6 a-blocks x 64 h];
                # transpose the six [128,64] blocks -> woT[i//2] row half.
                t, hb = i // 2, 64 * (i % 2)
                for g in range(2):
                    pt = ps_mm.tile([128, 512], f32, tag="mm", name="mm")
                    for a in range(3):
                        nc.tensor.transpose(
                            pt[0:64, a * 128:(a + 1) * 128],
                            wostage[i][:, (3 * g + a) * 64:(3 * g + a + 1) * 64],
                            ident[:])
                    evict(woT[t][hb:hb + 64, g * 384:(g + 1) * 384],
                          pt[0:64, 0:384])

            # ---- pair 0 (special: V projection + late z) ----
            # Filler order tracks DMA arrivals: QKT(1) first (its weights are
            # already transposed), W_V transposes last (rows land latest).
            in_startup[0] = False   # ps_sc now belongs to the score tiles
            E0 = E_sets[0]
            emit_score_pair(0, E0, 0)
            emit_score_pair(0, E0, 1)
            emit_qkt_chain(0, 1, 1)
            sprinkle()
            p0_fillers = [lambda w=w, c=c: emit_qkt_chain(1, w, c)
                          for w in range(2) for c in range(PC)]
            p0_fillers += [sprinkle, lambda: emit_w_transposes("v", wvT, 0)]
            fi = 0
            for j in range(2, ST):
                emit_score_pair(0, E0, j)
                if fi < len(p0_fillers):
                    p0_fillers[fi]()
                    fi += 1
            while fi < len(p0_fillers):
                p0_fillers[fi]()
                fi += 1
            emit_w_transposes("v", wvT, 1)
            for j in range(4):
                emit_v_tile(j)
            emit_z_chain(0, E0, 0, 0)
            emit_z_chain(0, E0, 0, 1)
            emit_score_pair(1, E_sets[1], 0)
            emit_score_pair(1, E_sets[1], 1)
            for j in range(4, ST):
                emit_v_tile(j)
            emit_score_pair(1, E_sets[1], 2)
            emit_score_pair(1, E_sets[1], 3)
            emit_z_chain(0, E0, 1, 0)
            emit_z_chain(0, E0, 1, 1)

            # ---- pairs 1..5 ----
            for t in range(1, NT):
                E_t = E_sets[t % 2]
                if t == 1:
                    # group-1 weight rows land last: transpose them here
                    fillers = [lambda: emit_w_transposes("q", wqT, 1),
                               lambda: emit_w_transposes("k", wkT, 1)]
                    fillers += [lambda w=w, c=c: emit_qkt_chain(2, w, c)
                                for w in range(2) for c in range(PC)]
                elif t + 1 < NT:
                    fillers = [lambda w=w, c=c, tt=t + 1: emit_qkt_chain(tt, w, c)
                               for w in range(2) for c in range(PC)]
                else:
                    fillers = []   # rows start after the loop: any evict
                                   # emitted before exp(j7) delays the
                                   # tri(j7) -> c1b dependency chain
                if t in (1, 2, 3):
                    fillers += [lambda i=i: emit_wo_transpose_group(i)
                                for i in range(4 * (t - 1), 4 * t)]
                fi = 0
                for j in range(4, ST):
                    emit_score_pair(t, E_t, j)
                    if j == 4:
                        emit_z_chain(t, E_t, 0, 0)
                    elif j == 5:
                        emit_z_chain(t, E_t, 0, 1)
                        if t + 1 == NT:
                            # last pair: first 256-col half of c1 as soon as
                            # its key blocks (j<=5) are exponentiated
                            emit_z_chain_q256(t, E_t, 0, 0)
                    elif t + 1 == NT and j == 6:
                        emit_z_chain_q256(t, E_t, 0, 1)
                    # out-proj fillers need z(c0) of pair 5: only after j==5
                    if (t + 1 < NT or j >= 6) and fi < len(fillers):
                        fillers[fi]()
                        fi += 1
                while fi < len(fillers):
                    fillers[fi]()
                    fi += 1
                # prefetch ALL of the next pair's j<=3 score tiles,
                # interleaved with this pair's c1 chains: the next pair's
                # tri-masks then precede these norms in the DVE queue, so
                # its z(c0) chains start unstalled at j==4
                if t + 1 < NT:
                    E_next = E_sets[(t + 1) % 2]
                    emit_score_pair(t + 1, E_next, 0)
                    emit_score_pair(t + 1, E_next, 1)
                    emit_z_chain(t, E_t, 1, 0)
                    emit_score_pair(t + 1, E_next, 2)
                    emit_score_pair(t + 1, E_next, 3)
                    emit_z_chain(t, E_t, 1, 1)
                else:
                    # last pair: c1b chains immediately (their exps are
                    # done), rows 2-3 give the PE work while the c1a norms
                    # drain, then rows 4-7 land as their ZT columns finish
                    emit_outproj_row(0, tail=True)
                    emit_outproj_row(1, tail=True)
                    emit_outproj_row(2, tail=True)
                    emit_outproj_row(3, tail=True)
                    emit_z_chain_q256(t, E_t, 1, 0)
                    emit_z_chain_q256(t, E_t, 1, 1)
                    for qj in range(4, ST):
                        emit_outproj_row(qj, tail=True, chunked=(qj >= 6))

            if debug:
                dpool = ctx.enter_context(tc.tile_pool(name="dpool", bufs=1))

                def dump(name, tile_ap):
                    fs = 1
                    for s_ in tile_ap.shape[1:]:
                        fs *= s_
                    f = dpool.tile([128, fs], f32, tag="d", name="d")
                    nc.vector.tensor_copy(f[:, 0:fs], tile_ap)
                    nc.sync.dma_start(dbg[name].ap(), f[:, 0:fs])

                for t in range(NT):
                    dump(f"dZT{t}", ZT[t][:])
                    dump(f"dQT{t}", QT[t][:])
                    dump(f"dKT{t}", KT[t][:])
                    dump(f"dwoT{t}", woT[t][:])
                for j in range(ST):
                    dump(f"dV{j}", V_sb[j][:])
                    # E set 1 holds pair 5's post-exp/mask values at the end
                    nc.sync.dma_start(dbg[f"dE{j}"].ap(), E_sets[1][j][:])

    nc.compile()
    return nc


_NC_CACHE = None


def _get_nc():
    global _NC_CACHE
    if _NC_CACHE is None:
        _NC_CACHE = build(N_CORES)
    return _NC_CACHE


def run(inputs, trace=False, **kwargs):
    nc = _get_nc()
    weights = {k: np.ascontiguousarray(np.asarray(inputs[k], dtype=np.float32))
               for k in ("W_Q", "W_K", "W_V", "W_O")}
    xs = np.ascontiguousarray(np.asarray(inputs["x"], dtype=np.float32))
    in_maps = [dict(weights, x=xs[b]) for b in range(B)]
    res = run_bass_kernel_spmd(nc, in_maps, core_ids=list(range(N_CORES)),
                               trace=trace, **kwargs)
    out = np.stack([np.asarray(res.results[b]["out"]) for b in range(B)], axis=0)
    return out.astype(np.float32), res


def kernel(**inputs) -> np.ndarray:
    out, _ = run(inputs, trace=False)
    return out
